# revision 13
# baseline (speedup 1.0000x reference)
"""Trainium2 Bass kernel for LocalDenseSynthesizerAttention (band C=63, H=4 heads).

Sharding: 8192 tokens (B=2 x T=4096 flattened) split contiguously across 8
cores (1024 tokens each).  Each core runs an identical program on its own
slice; batch-edge band masking and value halo padding are handled host-side
via per-core input data, so the program is uniform SPMD.

Band construction is fully on-chip (no DRAM staging round-trip):
  scores -> exp -> normalize (token-major pn [128, H*63], DVE)
  -> gpsimd local_scatter with a constant per-partition index tensor
     (idx[i, h*63+k] = 256*h + i + k), which skews pn into the dense band
     matrix sband [128 tokens, 4 heads x 256 window-rows] and zero-fills
     off-band positions in the same instruction
  -> S^T chunks via SBUF->SBUF XBAR dma transpose (two tiles per
     instruction: [128, 2048] -> [128, 16, 128]), or PE transposes for
     pairs listed in pe_t_pairs (balance knob DMA engines <-> PE).
All band matmuls keep base partition 0 (PE tile_position quadrants other
than (0,*) lock up the device).
"""

import numpy as np
import ml_dtypes

import concourse.bass as bass
import concourse.bacc as bacc
import concourse.mybir as mybir
import concourse.tile as tile
from concourse.ap import AP
from concourse import bass_utils

BF16 = mybir.dt.bfloat16
FP32 = mybir.dt.float32
I16 = mybir.dt.int16
NP_BF16 = ml_dtypes.bfloat16

B, T, NF = 2, 4096, 256
H, C, DK = 4, 63, 64
HALF = (C - 1) // 2  # 31
N_CORES = 8
TPC = (B * T) // N_CORES  # 1024 tokens per core
N_TILES = TPC // 128  # 8
VPAD = 1152  # parked value rows: tokens [-31, 1121) relative to core start
SW = 256  # per-head section width in the band buffer
SBW = H * SW  # 1024
NCH = SBW // 128  # 8 chunks of S^T
WPK = 1148  # w1T | w2T | w3T | woT | ident+pad

# tile-pairs whose S^T is built with PE transposes instead of the XBAR dma
# transpose (rebalances DMA-engine load onto the tensor engine)
PE_T_PAIRS = ()


def build_program(reps: int = 1, pe_t_pairs=PE_T_PAIRS):
    import contextlib

    nc = bacc.Bacc(
        "TRN2",
        target_bir_lowering=False,
        debug=False,
        enable_asserts=False,
        num_devices=N_CORES,
    )

    qT_d = nc.dram_tensor("qT", [NF, TPC], BF16, kind="ExternalInput").ap()
    vT_d = nc.dram_tensor("vT", [NF, VPAD], BF16, kind="ExternalInput").ap()
    wpack_d = nc.dram_tensor("wpack", [NF, WPK], BF16, kind="ExternalInput").ap()
    mask2_d = nc.dram_tensor("mask2", [128, 2 * C], BF16, kind="ExternalInput").ap()
    idx_d = nc.dram_tensor("idx", [128, H * C], I16, kind="ExternalInput").ap()
    outT_d = nc.dram_tensor("outT", [NF, TPC], BF16, kind="ExternalOutput").ap()

    with tile.TileContext(nc) as tc:
        with (
            tc.tile_pool(name="inp", bufs=1) as inp,
            tc.tile_pool(name="work", bufs=4) as work,
            tc.tile_pool(name="big_ps", bufs=1, space="PSUM") as big_ps,
            tc.tile_pool(name="sc_ps", bufs=2, space="PSUM") as sc_ps,
            tc.tile_pool(name="x_ps", bufs=2, space="PSUM") as x_ps,
            tc.tile_pool(name="tr_ps", bufs=2, space="PSUM") as tr_ps,
        ):
            # ---- persistent SBUF tensors --------------------------------
            qt_in = inp.tile([128, 2, TPC], BF16, tag="qt_in")
            vt_in = inp.tile([128, 2, VPAD], BF16, tag="vt_in")
            wall = inp.tile([128, 2, WPK], BF16, tag="wall")
            mask2 = inp.tile([128, 2 * C], BF16, tag="mask2")
            idxs = inp.tile([128, H * C], I16, tag="idxs")
            w1t = wall[:, :, 0:256]
            w2t = wall[:, :, 256:508]
            w3t = wall[:, :, 508:764]
            wot = wall[:, :, 764:1020]
            ident = wall[:, 0, 1020:1148]
            qtr = inp.tile([128, 2, TPC], BF16, tag="qtr")
            vpark = inp.tile([128, 9, NF], BF16, tag="vpark")
            xt = inp.tile([128, 2, TPC], BF16, tag="xt")
            outsb = inp.tile([128, 2, TPC], BF16, tag="outsb")
            # two consecutive tiles share one band tensor so a single
            # 2D-collapsible dma transpose covers both
            sband2 = [
                inp.tile([128, 2, SBW], BF16, tag=f"sband2_{i}", name=f"sband2_{i}")
                for i in range(2)
            ]
            sta2 = [
                inp.tile(
                    [128, 2, NCH, 128], BF16, tag=f"sta2_{i}", name=f"sta2_{i}"
                )
                for i in range(2)
            ]

            loop_ctx = (
                tc.For_i(0, reps, 1, hint_engines=(mybir.EngineType.PE,))
                if reps > 1
                else contextlib.nullcontext()
            )
            with loop_ctx:
                # ---- input DMAs (weights first; one instruction each) --
                nc.sync.dma_start(wall[:], wpack_d.rearrange("(c p) t -> p c t", p=128))
                nc.sync.dma_start(mask2[:], mask2_d)
                nc.sync.dma_start(idxs[:], idx_d)
                qT_r = qT_d.rearrange("(c p) t -> p c t", p=128)
                vT_r = vT_d.rearrange("(c p) t -> p c t", p=128)
                nc.sync.dma_start(qt_in[:], qT_r)
                nc.sync.dma_start(vt_in[:], vT_r)

                # ---- stage 1: qtr = relu(w1 @ queryT) ------------------
                for m in range(2):  # mega-tiles of 512 tokens
                    for mc in range(2):  # output feature chunk
                        ps = big_ps.tile([128, 512], FP32, tag="big")
                        for kc in range(2):
                            nc.tensor.matmul(
                                ps[:],
                                w1t[:, kc, mc * 128 : (mc + 1) * 128],
                                qt_in[:, kc, m * 512 : (m + 1) * 512],
                                start=(kc == 0),
                                stop=(kc == 1),
                            )
                        nc.scalar.activation(
                            qtr[:, mc, m * 512 : (m + 1) * 512],
                            ps[:],
                            mybir.ActivationFunctionType.Relu,
                        )

                # ---- stage 2: V = value @ w3.T parked at -31 offset ----
                for vp in range(5):  # pairs of V tiles share one PSUM bank
                    nv = 2 if vp < 4 else 1
                    ps = big_ps.tile([128, 512], FP32, tag="big")
                    for j in range(nv):
                        vt = 2 * vp + j
                        for kc in range(2):
                            nc.tensor.matmul(
                                ps[:, j * 256 : (j + 1) * 256],
                                vt_in[:, kc, vt * 128 : (vt + 1) * 128],
                                w3t[:, kc, :],
                                start=(kc == 0),
                                stop=(kc == 1),
                            )
                    if vp % 2 == 0:
                        nc.vector.tensor_copy(
                            vpark[:, 2 * vp : 2 * vp + nv, :],
                            ps[:, 0 : nv * 256].rearrange("p (a b) -> p a b", a=nv),
                        )
                    else:
                        nc.scalar.activation(
                            vpark[:, 2 * vp : 2 * vp + nv, :],
                            ps[:, 0 : nv * 256].rearrange("p (a b) -> p a b", a=nv),
                            mybir.ActivationFunctionType.Copy,
                        )

                # ---- per-tile band pipeline (software depth 2) ---------
                for t in range(N_TILES + 2):
                    if t < N_TILES:
                        # scores for tile t -> PSUM
                        sc = sc_ps.tile([128, H * C], FP32, tag="sc")
                        for kc in range(2):
                            nc.tensor.matmul(
                                sc[:],
                                qtr[:, kc, t * 128 : (t + 1) * 128],
                                w2t[:, kc, :],
                                start=(kc == 0),
                                stop=(kc == 1),
                            )
                        if t == 0 or t == N_TILES - 1:
                            moff = 0 if t == 0 else C
                            mb = AP(
                                mask2[:].tensor,
                                mask2[:].offset + moff,
                                [[2 * C, 128], [0, H], [1, C]],
                            )
                            nc.vector.tensor_add(
                                sc[:].rearrange("p (h c) -> p h c", h=H),
                                sc[:].rearrange("p (h c) -> p h c", h=H),
                                mb,
                            )
                        # exp, per-head denominators, normalize
                        expp = work.tile([128, H * C], BF16, tag="expp")
                        nc.scalar.activation(
                            expp[:], sc[:], mybir.ActivationFunctionType.Exp
                        )
                        den = work.tile([128, H], FP32, tag="den")
                        nc.vector.tensor_reduce(
                            den[:],
                            expp[:].rearrange("p (h c) -> p h c", h=H),
                            axis=mybir.AxisListType.X,
                            op=mybir.AluOpType.add,
                        )
                        rden = work.tile([128, H], FP32, tag="rden")
                        nc.vector.reciprocal(rden[:], den[:])
                        pn = work.tile([128, H * C], BF16, tag="pn")
                        rb = AP(
                            rden[:].tensor, rden[:].offset, [[H, 128], [1, H], [0, C]]
                        )
                        nc.vector.tensor_mul(
                            pn[:].rearrange("p (h c) -> p h c", h=H),
                            expp[:].rearrange("p (h c) -> p h c", h=H),
                            rb,
                        )
                        # skew pn into the dense band matrix (zero-fills rest)
                        pi = (t // 2) % 2
                        sb2 = sband2[pi]
                        nc.gpsimd.local_scatter(
                            sb2[:, t % 2, :],
                            pn[:],
                            idxs[:],
                            channels=128,
                            num_elems=SBW,
                            num_idxs=H * C,
                        )
                        if t // 2 in pe_t_pairs:
                            trp = tr_ps.tile([128, SBW], BF16, tag="trp")
                            for ch in range(NCH):
                                nc.tensor.transpose(
                                    trp[:, ch * 128 : (ch + 1) * 128],
                                    sb2[:, t % 2, ch * 128 : (ch + 1) * 128],
                                    ident,
                                )
                            if t % 2 == 0:
                                nc.vector.tensor_copy(
                                    sta2[pi][:, t % 2].rearrange("p c i -> p (c i)"),
                                    trp[:],
                                )
                            else:
                                nc.scalar.activation(
                                    sta2[pi][:, t % 2].rearrange("p c i -> p (c i)"),
                                    trp[:],
                                    mybir.ActivationFunctionType.Copy,
                                )
                        elif t % 2 == 1:
                            # both halves of the pair ready: one dma transpose
                            # for both tiles ([128, 2048] -> [128, 16, 128])
                            nc.scalar.dma_start_transpose(
                                sta2[pi][:].rearrange("p a c i -> p (a c) i"),
                                sb2[:],
                            )

                    if t >= 2:
                        s = t - 2
                        sta = sta2[(s // 2) % 2][:, s % 2]
                        xps = x_ps.tile([128, 256], FP32, tag="xv")
                        for h in range(H):
                            out_sl = xps[
                                64 * (h % 2) : 64 * (h % 2) + 64,
                                128 * (h // 2) : 128 * (h // 2) + 128,
                            ]
                            nc.tensor.matmul(
                                out_sl,
                                vpark[0:128, s, h * DK : (h + 1) * DK],
                                sta[0:128, 2 * h, :],
                                start=True,
                                stop=False,
                            )
                            nc.tensor.matmul(
                                out_sl,
                                vpark[0:62, s + 1, h * DK : (h + 1) * DK],
                                sta[0:62, 2 * h + 1, :],
                                start=False,
                                stop=True,
                            )
                        # one copy per tile: (h0,h1 | h2,h3) -> xt chunks
                        xdst = AP(
                            xt[:].tensor,
                            xt[:].offset + s * 128,
                            [[2 * TPC, 128], [TPC, 2], [1, 128]],
                        )
                        if s % 2 == 0:
                            nc.vector.tensor_copy(
                                xdst, xps[:].rearrange("p (a b) -> p a b", a=2)
                            )
                        else:
                            nc.scalar.activation(
                                xdst,
                                xps[:].rearrange("p (a b) -> p a b", a=2),
                                mybir.ActivationFunctionType.Copy,
                            )

                        # ---- out-proj per 512-token mega, interleaved --
                        if s % 4 == 3:
                            m = s // 4
                            outT_r = outT_d.rearrange("(c p) t -> p c t", p=128)
                            for mc in range(2):
                                ps = big_ps.tile([128, 512], FP32, tag="big")
                                for kc in range(2):
                                    nc.tensor.matmul(
                                        ps[:],
                                        wot[:, kc, mc * 128 : (mc + 1) * 128],
                                        xt[:, kc, m * 512 : (m + 1) * 512],
                                        start=(kc == 0),
                                        stop=(kc == 1),
                                    )
                                if mc == 0:
                                    nc.vector.tensor_copy(
                                        outsb[:, mc, m * 512 : (m + 1) * 512], ps[:]
                                    )
                                else:
                                    nc.scalar.activation(
                                        outsb[:, mc, m * 512 : (m + 1) * 512],
                                        ps[:],
                                        mybir.ActivationFunctionType.Copy,
                                    )
                            nc.sync.dma_start(
                                outT_r[:, :, m * 512 : (m + 1) * 512],
                                outsb[:, :, m * 512 : (m + 1) * 512],
                            )

    nc.compile()
    return nc


def make_inputs(query, value, w1, w2, w3, w_out):
    """Host-side shard/transpose/cast. Returns per-core in_maps."""
    fq = np.asarray(query, np.float32).reshape(B * T, NF)
    fv = np.asarray(value, np.float32).reshape(B * T, NF)
    wpack = np.zeros((NF, WPK), np.float32)
    wpack[:, 0:256] = np.asarray(w1, np.float32).T
    wpack[:, 256:508] = np.asarray(w2, np.float32).T
    wpack[:, 508:764] = np.asarray(w3, np.float32).T
    wpack[:, 764:1020] = np.asarray(w_out, np.float32).T
    wpack[0:128, 1020:1148] = np.eye(128, dtype=np.float32)
    wpack = wpack.astype(NP_BF16)

    # constant scatter indices: idx[i, h*63+k] = 256*h + i + k
    ii = np.arange(128)[:, None]
    hh = np.repeat(np.arange(H), C)[None, :]
    kk = np.tile(np.arange(C), H)[None, :]
    idx = (SW * hh + ii + kk).astype(np.int16)

    in_maps = []
    k = np.arange(C)
    for c in range(N_CORES):
        t0 = c * TPC
        b = (c * TPC) // T
        b0, b1 = b * T, (b + 1) * T
        qT = np.ascontiguousarray(fq[t0 : t0 + TPC].T).astype(NP_BF16)
        # parked value rows: global tokens [t0-31, t0-31+VPAD), zero outside
        vrows = np.zeros((VPAD, NF), np.float32)
        lo = t0 - HALF
        s0, s1 = max(lo, b0), min(lo + VPAD, b1)
        vrows[s0 - lo : s1 - lo] = fv[s0:s1]
        vT = np.ascontiguousarray(vrows.T).astype(NP_BF16)
        # additive band masks for first/last tile (batch edges only),
        # head-independent: [128, 2*C] (first tile | last tile)
        mask2 = np.zeros((128, 2 * C), np.float32)
        g = t0 + np.arange(128)[:, None]
        bad = (g + k - HALF < b0) | (g + k - HALF >= b1)
        mask2[:, :C] = np.where(bad, -30000.0, 0.0)
        g = t0 + (N_TILES - 1) * 128 + np.arange(128)[:, None]
        bad = (g + k - HALF < b0) | (g + k - HALF >= b1)
        mask2[:, C:] = np.where(bad, -30000.0, 0.0)
        in_maps.append(
            {
                "qT": qT,
                "vT": vT,
                "wpack": wpack,
                "mask2": mask2.astype(NP_BF16),
                "idx": idx,
            }
        )
    return in_maps


_NC_CACHE = None


def kernel(query, key, value, mask, w1, w2, w3, w_out):
    global _NC_CACHE
    if _NC_CACHE is None:
        _NC_CACHE = build_program()
    nc = _NC_CACHE
    in_maps = make_inputs(query, value, w1, w2, w3, w_out)
    res = bass_utils.run_bass_kernel_spmd(nc, in_maps, core_ids=list(range(N_CORES)))
    outs = []
    for c in range(N_CORES):
        outT = res.results[c]["outT"]  # (256, 1024)
        outs.append(np.ascontiguousarray(outT.T))
    full = np.concatenate(outs, axis=0)  # (8192, 256)
    return full.reshape(B, T, NF).astype(np.float32)
